# revision 25
# baseline (speedup 1.0000x reference)
"""GCN (4-layer, PyG GCNConv-style) Trainium2 Bass kernel, SPMD over 8 NeuronCores.

Strategy
--------
Nodes are sharded round-robin-free (contiguous blocks) across 8 cores; edges are
partitioned by destination node.  Per layer:
  1. transform: each core computes h_t = dinv * (h_relu @ W) for its own rows
     (PE matmul, bf16), writes them to an HBM staging buffer.
  2. AllGather the staged rows so each core holds the full [N, 128]-padded
     bf16 feature table in HBM.
  3. gather: per-edge source rows fetched with dma_gather (256B descriptors).
  4. scatter-add: one-hot segment matrices S (fp8, host-precomputed, streamed
     from HBM) contract gathered message tiles on the TensorEngine into PSUM,
     accumulating per-destination sums; epilogue applies dinv[dst], bias, relu.
Final classifier + log_softmax computed per-core on its own rows.

All edge sorting / padding / one-hot construction happens on the host in numpy
inside kernel().  The dma_gather int16 index limit (32767) is handled by
splitting messages into two halves by source row (< 32768 / >= 32768) with a
re-based source view for the second half.

Perf-critical settings (measured on axon-tunneled TRN2):
  * num_swdge_queues=4 with gather calls round-robined over queue 0..3 —
    SWDGE descriptor *generation* on the Q7 is the gather bottleneck
    (~11.3 ns/desc on one queue); 4 queues generate concurrently (~4x).
  * single_packet=False on dma_gather — ~2x faster end-to-end.
  * dma_gather calls capped at 1024 indices: the per-queue descriptor ring
    holds 1024 descriptors; larger calls wedge or kill the device
    (NRT_EXEC_UNIT_UNRECOVERABLE), independent of dynamic_dma_scratch_size.
Measured ~2.2-2.8 ms per forward pass (repeat-delta timing), rel err 6e-4.
"""

import math
import os
import sys

import numpy as np

sys.path.insert(0, "/opt/trn_rl_repo")

import ml_dtypes  # noqa: E402

NCORES = 8
TILE = 128
D = 96
HALF = 32768  # int16-addressable row limit for dma_gather indices
ST_TILES = 4  # tiles per supertile (one gather call pair per supertile)
N_LAYERS = 4
SINGLE_PACKET = False
MSG_BUFS = 2
GATHER_CALL_IDX = 1024  # <=1024: SWDGE ring capacity
NUM_QUEUES = 4
DMA_SCRATCH = 16384  # descriptor carveout: ring capacity = this/16 descs



def _ceil_div(a, b):
    return -(-a // b)


# ---------------------------------------------------------------------------
# Host-side preprocessing
# ---------------------------------------------------------------------------


class Plan:
    """Shared (core-independent) structure + per-core data arrays."""

    pass


def _prep(x, edge_index):
    """Build the shared chunk structure and per-core input arrays."""
    x = np.asarray(x, dtype=np.float32)
    edge_index = np.asarray(edge_index, dtype=np.int64)
    N, d_in = x.shape
    assert d_in == D
    NPC = N // NCORES
    assert NPC * NCORES == N
    NT = _ceil_div(NPC, TILE)
    NTP = NT * TILE
    NST = _ceil_div(NT, ST_TILES)

    # self-loop messages are computed on-device as a W^T @ (h*dinv) matmul
    # accumulated into the same PSUM as the edge chunks, so the gather
    # structure only carries the real edges.
    src_all = edge_index[0]
    dst_all = edge_index[1]
    M = src_all.shape[0]

    deg = (np.bincount(dst_all, minlength=N) + 1).astype(np.float32)
    dinv = (1.0 / np.sqrt(deg)).astype(np.float32)

    core = dst_all // NPC
    tl = (dst_all % NPC) // TILE
    hb = (src_all >= HALF).astype(np.int64)
    gid = (core * NT + tl) * 2 + hb
    order = np.argsort(gid, kind="stable")
    gsrc = src_all[order]
    gdst = dst_all[order]
    gid_s = gid[order]

    counts = np.bincount(gid, minlength=NCORES * NT * 2).reshape(NCORES, NT, 2)
    # chunks per (tile, half): max over cores so the instruction stream is shared
    KA = _ceil_div(counts[:, :, 0].max(axis=0), TILE)  # [NT]
    KB = _ceil_div(counts[:, :, 1].max(axis=0), TILE)  # [NT]
    K = KA + KB

    # supertile structure ---------------------------------------------------
    st_tiles = [list(range(s * ST_TILES, min((s + 1) * ST_TILES, NT))) for s in range(NST)]

    # global chunk ids: per supertile: A-chunks tile-major, then B-chunks
    gbaseA = np.zeros(NT, dtype=np.int64)
    gbaseB = np.zeros(NT, dtype=np.int64)
    # position of chunk within its supertile's msg buffer
    lbaseA = np.zeros(NT, dtype=np.int64)
    lbaseB = np.zeros(NT, dtype=np.int64)
    st_of_tile = np.zeros(NT, dtype=np.int64)
    st_chunk_off = np.zeros(NST, dtype=np.int64)  # global chunk id of supertile start
    st_nchunks = np.zeros(NST, dtype=np.int64)
    g = 0
    for s, tiles in enumerate(st_tiles):
        st_chunk_off[s] = g
        off = 0
        for t in tiles:
            st_of_tile[t] = s
            gbaseA[t] = g
            lbaseA[t] = off
            g += KA[t]
            off += KA[t]
        for t in tiles:
            gbaseB[t] = g
            lbaseB[t] = off
            g += KB[t]
            off += KB[t]
        st_nchunks[s] = off
    TOTCH = g

    # gather calls are per (tile, half) group; idx columns follow global
    # chunk order, so chunk g owns idx columns [g*8, (g+1)*8).
    TOTIDX16 = TOTCH * (TILE // 16)  # idx array free-dim length (int16 cols)

    # ---------------------------------------------------------------- per-msg
    # position within (core, tile, half) group
    gstart = np.zeros(NCORES * NT * 2 + 1, dtype=np.int64)
    np.cumsum(np.bincount(gid_s, minlength=NCORES * NT * 2), out=gstart[1:])
    pos = np.arange(M, dtype=np.int64) - gstart[gid_s]

    m_core = gid_s // (NT * 2)
    m_tile = (gid_s // 2) % NT
    m_half = gid_s % 2
    m_chunk_in_group = pos // TILE
    m_part = pos % TILE
    m_gchunk = np.where(m_half == 0, gbaseA[m_tile], gbaseB[m_tile]) + m_chunk_in_group
    m_dstloc = gdst - (m_core * NPC + m_tile * TILE)
    m_idx16 = np.where(m_half == 0, gsrc, gsrc - HALF).astype(np.int16)
    m_col = m_gchunk * (TILE // 16) + (pos % TILE) // 16
    m_row16 = pos % 16

    # ---------------------------------------------------------------- arrays
    plan = Plan()
    plan.N, plan.NPC, plan.NT, plan.NTP, plan.NST = N, NPC, NT, NTP, NST
    plan.st_tiles = st_tiles
    plan.KA, plan.KB, plan.K = KA, KB, K
    plan.gbaseA, plan.gbaseB = gbaseA, gbaseB
    plan.st_chunk_off, plan.st_nchunks = st_chunk_off, st_nchunks
    plan.lbaseA, plan.lbaseB = lbaseA, lbaseB
    plan.TOTCH, plan.TOTIDX16 = TOTCH, TOTIDX16
    plan.dinv = dinv
    plan.iota = np.broadcast_to(
        np.arange(TILE, dtype=ml_dtypes.bfloat16)[None, :], (TILE, TILE)
    ).copy()

    # full dinv*x table for layer-0 gathers (replicated input; cols 96:128 pad)
    xd = x * dinv[:, None]
    xtab = np.zeros((N, TILE), dtype=ml_dtypes.bfloat16)
    xtab[:, :D] = xd.astype(ml_dtypes.bfloat16)
    plan.xtab = xtab

    per_core = []
    for c in range(NCORES):
        sel = m_core == c
        # dst-column per (chunk, slot) for on-device one-hot S generation:
        # S[p, g*128+c] = (dstv[p, g] == c); padded slots get -1 (no match)
        dstv = np.full((TILE, TOTCH), -1.0, dtype=ml_dtypes.bfloat16)
        dstv[m_part[sel], m_gchunk[sel]] = m_dstloc[sel].astype(
            ml_dtypes.bfloat16
        )
        # idx [128, TOTIDX16] int16 (wrapped by 16, replicated across 8 groups)
        # padding entries stay 0 (gather row 0; S column is zero there).
        # NOTE: negative-index trimming wedges the device with this ring
        # setup (decode-side ring-space accounting uses the untrimmed count)
        idx16 = np.zeros((16, TOTIDX16), dtype=np.int16)
        idx16[m_row16[sel], m_col[sel]] = m_idx16[sel]
        idx = np.tile(idx16, (8, 1))
        # xTd [96, NTP] bf16: own rows of dinv*x, transposed (self-loop add)
        xTd = np.zeros((D, NTP), dtype=ml_dtypes.bfloat16)
        xTd[:, :NPC] = xd[c * NPC : (c + 1) * NPC].T.astype(ml_dtypes.bfloat16)
        # dinvT replicated [96, NTP]
        dinvT = np.ones((D, NTP), dtype=np.float32)
        dinvT[:, :NPC] = dinv[c * NPC : (c + 1) * NPC][None, :]
        # dinv per own row, tile-column layout [128, NT]
        downv = np.ones((TILE, NT), dtype=np.float32)
        dv = dinv[c * NPC : (c + 1) * NPC]
        dvp = np.zeros(NTP, dtype=np.float32)
        dvp[:NPC] = dv
        downv[:, :] = dvp.reshape(NT, TILE).T
        per_core.append(dict(dstv=dstv, idx=idx, xTd=xTd, dinvT=dinvT, dinvown=downv))
    plan.per_core = per_core
    return plan


# ---------------------------------------------------------------------------
# Bass program builder
# ---------------------------------------------------------------------------


def _build(plan, repeats=1, skip=frozenset()):
    import concourse.bass as bass
    import concourse.bacc as bacc
    import concourse.mybir as mybir
    import concourse.tile as tile

    f32 = mybir.dt.float32
    bf16 = mybir.dt.bfloat16
    fp8 = mybir.dt.float8e4
    i16 = mybir.dt.int16
    AF = mybir.ActivationFunctionType
    ALU = mybir.AluOpType

    N, NPC, NT, NTP, NST = plan.N, plan.NPC, plan.NT, plan.NTP, plan.NST
    TOTCH, TOTIDX16 = plan.TOTCH, plan.TOTIDX16
    KA, KB = plan.KA, plan.KB

    nc = bacc.Bacc(
        None,
        target_bir_lowering=False,
        num_swdge_queues=NUM_QUEUES,
        dynamic_dma_scratch_size=DMA_SCRATCH,
    )

    xtab_p = nc.declare_dram_parameter("xtab", [N, TILE], bf16, isOutput=False)
    xTd_p = nc.declare_dram_parameter("xTd", [D, NTP], bf16, isOutput=False)
    idx_p = nc.declare_dram_parameter("idx", [TILE, TOTIDX16], i16, isOutput=False)
    dstv_p = nc.declare_dram_parameter("dstv", [TILE, TOTCH], bf16, isOutput=False)
    iota_p = nc.declare_dram_parameter("iota", [TILE, TILE], bf16, isOutput=False)
    dinvT_p = nc.declare_dram_parameter("dinvT", [D, NTP], f32, isOutput=False)
    dinvown_p = nc.declare_dram_parameter("dinvown", [TILE, NT], f32, isOutput=False)
    biasT_p = nc.declare_dram_parameter("biasT", [D, N_LAYERS], f32, isOutput=False)
    brep_p = nc.declare_dram_parameter("brep", [TILE, 4], f32, isOutput=False)
    w_p = nc.declare_dram_parameter("W", [D, N_LAYERS * D], f32, isOutput=False)
    wl_p = nc.declare_dram_parameter("Wl", [D, 4], f32, isOutput=False)
    out_p = nc.declare_dram_parameter("out", [NPC, 4], f32, isOutput=True)

    replica_groups = [list(range(NCORES))]

    with tile.TileContext(nc) as tc:
        with (
            tc.tile_pool(name="persist", bufs=1) as persist,
            tc.tile_pool(name="hrelu", bufs=2) as hrelu_pool,
            tc.tile_pool(name="hdinv", bufs=2) as hdinv_pool,
            tc.tile_pool(name="msg", bufs=MSG_BUFS) as msg_pool,
            tc.tile_pool(name="spool", bufs=2) as s_pool,
            tc.tile_pool(name="tmp", bufs=3) as tmp_pool,
            tc.tile_pool(name="stage", bufs=3) as stage_pool,
            tc.tile_pool(name="small", bufs=3) as small_pool,
            tc.tile_pool(name="psA", bufs=6, space="PSUM") as psA_pool,
            tc.tile_pool(name="ps3", bufs=2, space="PSUM") as ps3_pool,
            tc.tile_pool(name="dram", bufs=2, space="DRAM") as dram_pool,
        ):
            # ------------------------------------------------- persistent loads
            xTd_sb = persist.tile([D, NTP], bf16, tag="xTd")
            nc.sync.dma_start(xTd_sb[:], xTd_p[:])
            idx_sb = persist.tile([TILE, TOTIDX16], i16, tag="idx")
            nc.sync.dma_start(idx_sb[:], idx_p[:])
            dinvT_sb = persist.tile([D, NTP], f32, tag="dinvT")
            nc.sync.dma_start(dinvT_sb[:], dinvT_p[:])
            dinvown_sb = persist.tile([TILE, NT], f32, tag="dinvown")
            nc.sync.dma_start(dinvown_sb[:], dinvown_p[:])
            biasT_sb = persist.tile([D, N_LAYERS], f32, tag="biasT")
            nc.sync.dma_start(biasT_sb[:], biasT_p[:])
            brep_sb = persist.tile([TILE, 4], f32, tag="brep")
            nc.sync.dma_start(brep_sb[:], brep_p[:])
            dstv_sb = persist.tile([TILE, TOTCH], bf16, tag="dstv")
            nc.sync.dma_start(dstv_sb[:], dstv_p[:])
            iota_sb = persist.tile([TILE, TILE], bf16, tag="iota")
            nc.sync.dma_start(iota_sb[:], iota_p[:])
            w_sb = persist.tile([D, N_LAYERS * D], bf16, tag="W")
            nc.gpsimd.dma_start(w_sb[:], w_p[:])
            wl_sb = persist.tile([D, 4], bf16, tag="Wl")
            nc.gpsimd.dma_start(wl_sb[:], wl_p[:])

            def transform_stage(t, layer, src_T, ag_in):
                """h_t tile = (src_T[:, t] @ W[layer]) * dinv, staged to ag_in."""
                w = min(TILE, NPC - t * TILE)
                ps3 = ps3_pool.tile([TILE, D], f32, tag="ps3")
                nc.tensor.matmul(
                    ps3[:w],
                    src_T[:, t * TILE : t * TILE + w],
                    w_sb[:, layer * D : (layer + 1) * D],
                    start=True,
                    stop=True,
                )
                # pad cols 96:128 stay garbage: gathered but never read
                # (chunk matmuls consume msg3[:, j, 0:D] only)
                st = stage_pool.tile([TILE, TILE], bf16, tag="stage")
                nc.scalar.activation(
                    st[:w, 0:D], ps3[:w], AF.Copy, scale=dinvown_sb[:w, t : t + 1]
                )
                nc.sync.dma_start(ag_in[t * TILE : t * TILE + w, :], st[:w])

            def allgather(ag_in, ag_out):
                if "ag" in skip:
                    for r in range(NCORES):
                        nc.sync.dma_start(ag_out[r * NPC : (r + 1) * NPC, :], ag_in[:])
                else:
                    nc.gpsimd.collective_compute(
                        "AllGather",
                        ALU.bypass,
                        replica_groups=replica_groups,
                        ins=[ag_in[:]],
                        outs=[ag_out[:]],
                    )

            def classifier(t, src_T):
                w = min(TILE, NPC - t * TILE)
                psf = ps3_pool.tile([TILE, D], f32, tag="ps3")
                nc.tensor.matmul(
                    psf[:w, 0:4],
                    src_T[:, t * TILE : t * TILE + w],
                    wl_sb[:],
                    start=True,
                    stop=True,
                )
                xb = small_pool.tile([TILE, 4], f32, tag="xb")
                nc.vector.tensor_tensor(xb[:w], psf[:w, 0:4], brep_sb[:w], ALU.add)
                negm = small_pool.tile([TILE, 1], f32, tag="negm")
                nc.vector.tensor_reduce(
                    negm[:w], xb[:w], mybir.AxisListType.X, ALU.max, negate=True
                )
                ex = small_pool.tile([TILE, 4], f32, tag="ex")
                sumexp = small_pool.tile([TILE, 1], f32, tag="sumexp")
                nc.scalar.activation(
                    ex[:w], xb[:w], AF.Exp, bias=negm[:w], accum_out=sumexp[:w]
                )
                lse = small_pool.tile([TILE, 1], f32, tag="lse")
                nc.scalar.activation(lse[:w], sumexp[:w], AF.Ln)
                shift = small_pool.tile([TILE, 1], f32, tag="shift")
                nc.vector.tensor_sub(shift[:w], negm[:w], lse[:w])
                outt = small_pool.tile([TILE, 4], f32, tag="outt")
                nc.vector.tensor_scalar_add(outt[:w], xb[:w], shift[:w])
                nc.sync.dma_start(out_p[t * TILE : t * TILE + w, :], outt[:w])

            s_live = "sdma" not in skip

            def gen_S(s):
                """One-hot S for supertile s, on DVE from dstv/iota broadcasts.
                S is layer-independent; callers issue this one supertile ahead
                so the in-order DVE queue never stalls matmuls on it."""
                nch = int(plan.st_nchunks[s])
                goff = int(plan.st_chunk_off[s])
                s_sb = s_pool.tile([TILE, nch * TILE], fp8, tag="spool")
                s3 = s_sb[:].rearrange("p (c e) -> p c e", e=TILE)
                ngen = nch if s_live else 1
                nc.vector.tensor_tensor(
                    s3[:, 0:ngen, :],
                    dstv_sb[:, goff : goff + ngen]
                    .unsqueeze(-1)
                    .broadcast_to([TILE, ngen, TILE]),
                    iota_sb[:].unsqueeze(1).broadcast_to([TILE, ngen, TILE]),
                    ALU.is_equal,
                )
                return s_sb

            for rep in range(repeats):
              # layer 0 gathers straight from the host-prepared dinv*x table;
              # W0 is applied after aggregation (linearity), so there is no
              # transform/stage/AllGather prologue at all.
              ag_out = None
              cur_D = None
              s_next = gen_S(0)
              for layer in range(N_LAYERS):
                last = layer == N_LAYERS - 1
                new_T = hrelu_pool.tile([D, NTP], bf16, tag="hrelu")
                if not last:
                    new_D = hdinv_pool.tile([D, NTP], bf16, tag="hdinv")
                    ag_in2 = dram_pool.tile([NPC, TILE], bf16, tag="ag_in")
                    ag_out2 = dram_pool.tile(
                        [N, TILE],
                        bf16,
                        tag="ag_out",
                        addr_space="Local" if "ag" in skip else "Shared",
                    )
                gq = [0]
                for s, tiles in enumerate(plan.st_tiles):
                    nch = int(plan.st_nchunks[s])
                    goff = int(plan.st_chunk_off[s])
                    msg = msg_pool.tile([TILE, nch * TILE], bf16, tag="msg")
                    msg3 = msg[:].rearrange("p (c e) -> p c e", e=TILE)
                    s_sb = s_next
                    g_live = "gather" not in skip
                    # pre-generate the next supertile's S (same for all layers)
                    if s + 1 < plan.NST:
                        s_next = gen_S(s + 1)
                    elif not last or rep + 1 < repeats:
                        s_next = gen_S(0)
                    # gather calls split to <=1024 indices per call (SWDGE
                    # ring capacity)
                    GMAX = GATHER_CALL_IDX
                    table = xtab_p if layer == 0 else ag_out
                    if not g_live:
                        nc.gpsimd.dma_gather(
                            msg3[:, 0:1, :], table[:],
                            idx_sb[:, goff * 8 : goff * 8 + 8],
                            num_idxs=TILE, num_idxs_reg=TILE, elem_size=TILE,
                        )
                    for t in tiles:
                        for h in range(2):
                            k = int((KA if h == 0 else KB)[t])
                            if k == 0 or not g_live:
                                continue
                            n = k * TILE
                            gchunk0 = int((plan.gbaseA if h == 0 else plan.gbaseB)[t])
                            lbase = int((plan.lbaseA if h == 0 else plan.lbaseB)[t])
                            coloff = gchunk0 * 8
                            in_ap = table[:] if h == 0 else table[HALF:N, :]
                            for c0 in range(0, n, GMAX):
                                nn = min(GMAX, n - c0)
                                out_ap = msg3[
                                    :,
                                    lbase + c0 // TILE : lbase + (c0 + nn) // TILE,
                                    :,
                                ]
                                nc.gpsimd.dma_gather(
                                    out_ap,
                                    in_ap,
                                    idx_sb[
                                        :, coloff + c0 // 16 : coloff + (c0 + nn) // 16
                                    ],
                                    num_idxs=nn,
                                    num_idxs_reg=nn,
                                    elem_size=TILE,
                                    queue_num=gq[0] % NUM_QUEUES,
                                    single_packet=SINGLE_PACKET,
                                )
                                gq[0] += 1
                    # per-tile accumulation, tile-major chunk order; the
                    # self-loop term dinv_i^2 (h W)_i enters the same PSUM as
                    # a W^T @ (h*dinv) matmul (contraction over features)
                    for t in tiles:
                        kA, kB = int(KA[t]), int(KB[t])
                        lA = int(plan.gbaseA[t] - goff)
                        lB = int(plan.gbaseB[t] - goff)
                        locs = [lA + j for j in range(kA)] + [lB + j for j in range(kB)]
                        if "mm" in skip:
                            locs = locs[:1]
                        psA = psA_pool.tile([D, TILE], f32, tag="psA")
                        if layer > 0:
                            nc.tensor.matmul(
                                psA[:],
                                w_sb[:, layer * D : (layer + 1) * D],
                                cur_D[:, t * TILE : (t + 1) * TILE],
                                start=True,
                                stop=(len(locs) == 0),
                            )
                        for ji, j in enumerate(locs):
                            nc.tensor.matmul(
                                psA[:],
                                msg3[:, j if g_live else 0, 0:D],
                                s_sb[:, (j if s_live else 0) * TILE : ((j if s_live else 0) + 1) * TILE],
                                start=(layer == 0 and ji == 0),
                                stop=(ji == len(locs) - 1),
                            )
                        if layer == 0:
                            # agg = sum dinv_src*x_src (+ self-loop dinv_i*x_i),
                            # then apply W0 (commuted past the aggregation)
                            agg = tmp_pool.tile([D, TILE], bf16, tag="agg")
                            nc.vector.tensor_tensor(
                                agg[:],
                                psA[:],
                                xTd_sb[:, t * TILE : (t + 1) * TILE],
                                ALU.add,
                            )
                            psA = psA_pool.tile([D, TILE], f32, tag="psA")
                            nc.tensor.matmul(
                                psA[:],
                                w_sb[:, 0:D],
                                agg[:],
                                start=True,
                                stop=True,
                            )
                        tmp = tmp_pool.tile([D, TILE], f32, tag="tmp")
                        nc.vector.tensor_tensor(
                            tmp[:], psA[:], dinvT_sb[:, t * TILE : (t + 1) * TILE], ALU.mult
                        )
                        nc.scalar.activation(
                            new_T[:, t * TILE : (t + 1) * TILE],
                            tmp[:],
                            AF.Relu,
                            bias=biasT_sb[:, layer : layer + 1],
                        )
                        if not last:
                            nc.vector.tensor_tensor(
                                new_D[:, t * TILE : (t + 1) * TILE],
                                new_T[:, t * TILE : (t + 1) * TILE],
                                dinvT_sb[:, t * TILE : (t + 1) * TILE],
                                ALU.mult,
                            )
                            transform_stage(t, layer + 1, new_T, ag_in2)
                        else:
                            classifier(t, new_T)
                if not last:
                    allgather(ag_in2, ag_out2)
                    ag_out = ag_out2
                    cur_T, cur_D = new_T, new_D
                else:
                    cur_T = new_T

    nc.compile()
    return nc


# ---------------------------------------------------------------------------
# in_maps assembly
# ---------------------------------------------------------------------------


def _in_maps(plan, W0, b0, W1, b1, W2, b2, W3, b3, Wl, bl):
    Ws = np.concatenate(
        [np.asarray(w, np.float32) for w in (W0, W1, W2, W3)], axis=1
    )  # [96, 4*96]
    biasT = np.stack(
        [np.asarray(b, np.float32) for b in (b0, b1, b2, b3)], axis=1
    )  # [96, 4]
    brep = np.tile(np.asarray(bl, np.float32)[None, :], (TILE, 1))  # [128, 4]
    wl = np.asarray(Wl, np.float32)
    maps = []
    for c in range(NCORES):
        pc = plan.per_core[c]
        maps.append(
            {
                "xtab": plan.xtab,
                "xTd": pc["xTd"],
                "idx": pc["idx"],
                "dstv": pc["dstv"],
                "iota": plan.iota,
                "dinvT": pc["dinvT"],
                "dinvown": pc["dinvown"],
                "biasT": biasT,
                "brep": brep,
                "W": Ws,
                "Wl": wl,
                "out": np.zeros((plan.NPC, 4), np.float32),
            }
        )
    return maps


# ---------------------------------------------------------------------------
# public entry point
# ---------------------------------------------------------------------------

_CACHE = {}


def _get_compiled(plan):
    return _build(plan)


def kernel(x, edge_index, W0, b0, W1, b1, W2, b2, W3, b3, Wl, bl):
    from concourse.bass_utils import run_bass_kernel_spmd

    x = np.asarray(x, np.float32)
    edge_index = np.asarray(edge_index, np.int64)
    plan = _prep(x, edge_index)
    nc = _get_compiled(plan)
    in_maps = _in_maps(plan, W0, b0, W1, b1, W2, b2, W3, b3, Wl, bl)
    res = run_bass_kernel_spmd(nc, in_maps, core_ids=list(range(NCORES)))
    out = np.concatenate([res.results[c]["out"] for c in range(NCORES)], axis=0)
    return out.astype(np.float32)



# revision 27
# speedup vs baseline: 1.1701x; 1.1701x over previous
"""GCN (4-layer, PyG GCNConv-style) Trainium2 Bass kernel, SPMD over 8 NeuronCores.

Strategy
--------
Nodes are sharded in contiguous blocks across 8 cores; edges are partitioned by
destination node.  Pipeline per layer (supertile = 4 dst tiles):
  * gather: per-edge source rows fetched with dma_gather (256B descriptors,
    4 SWDGE queues).  Layer 0 gathers straight from a host-prepared dinv*x
    table (a replicated input) and W0 is applied AFTER aggregation
    (linearity commute) - no transform/stage/AllGather prologue exists.
  * scatter-add: one-hot S chunks are GENERATED ON DEVICE (one DVE is_equal
    per supertile from a [128, TOTCH] bf16 dst-column table against an iota
    row, hoisted one supertile ahead so the in-order DVE queue never blocks
    matmuls); S contracts gathered message chunks on the PE into PSUM.  The
    self-loop term enters the same PSUM as a W^T @ (h*dinv) matmul.
  * epilogue: dinv[dst] scale + bias + relu; then the NEXT layer's transform
    (h @ W * dinv, PE) for the same tiles is staged immediately to the
    AllGather input so the shared-output AllGather (~45-50us on-chip) fires
    right after the last supertile; layer-3 tiles run the classifier
    (log_softmax) instead.
Messages split into halves by source row (< 32768 / >= 32768) with a re-based
source view for the second half (dma_gather int16 index limit).

Perf notes (measured, axon-tunneled TRN2, interleaved repeat-delta timing):
  * dma_gather steady state ~1.4-1.8ns per 256B msg (HBM random-read bound,
    NOT desc-gen bound); kernel achieves ~2.0ns/slot including contention.
  * >1024-idx calls do work with dynamic_dma_scratch_size scaled up, but are
    perf-neutral; single_packet and queue count >=4 likewise.
  * negative-index trimming (trailing -1 idxs) WEDGES the device unless
    num_idxs_reg carries the per-core post-trim count (decode-side ring
    accounting reads the register, gen reads the trimmed static count).
  * staged-table pad columns 96:128 are never read by consumers - garbage OK.
Measured ~1.55-1.8 ms per forward pass, rel err 6e-4 (was 2.2-2.7 ms).
"""

import math
import os
import sys

import numpy as np

sys.path.insert(0, "/opt/trn_rl_repo")

import ml_dtypes  # noqa: E402

NCORES = 8
TILE = 128
D = 96
HALF = 32768  # int16-addressable row limit for dma_gather indices
ST_TILES = 4  # tiles per supertile (one gather call pair per supertile)
N_LAYERS = 4
SINGLE_PACKET = False
MSG_BUFS = 3
GATHER_CALL_IDX = 1024  # <=1024: SWDGE ring capacity
NUM_QUEUES = 4
DMA_SCRATCH = 16384  # descriptor carveout: ring capacity = this/16 descs
HOIST_S = True  # pre-generate next supertile S one step ahead on DVE



def _ceil_div(a, b):
    return -(-a // b)


# ---------------------------------------------------------------------------
# Host-side preprocessing
# ---------------------------------------------------------------------------


class Plan:
    """Shared (core-independent) structure + per-core data arrays."""

    pass


def _prep(x, edge_index):
    """Build the shared chunk structure and per-core input arrays."""
    x = np.asarray(x, dtype=np.float32)
    edge_index = np.asarray(edge_index, dtype=np.int64)
    N, d_in = x.shape
    assert d_in == D
    NPC = N // NCORES
    assert NPC * NCORES == N
    NT = _ceil_div(NPC, TILE)
    NTP = NT * TILE
    NST = _ceil_div(NT, ST_TILES)

    # self-loop messages are computed on-device as a W^T @ (h*dinv) matmul
    # accumulated into the same PSUM as the edge chunks, so the gather
    # structure only carries the real edges.
    src_all = edge_index[0]
    dst_all = edge_index[1]
    M = src_all.shape[0]

    deg = (np.bincount(dst_all, minlength=N) + 1).astype(np.float32)
    dinv = (1.0 / np.sqrt(deg)).astype(np.float32)

    core = dst_all // NPC
    tl = (dst_all % NPC) // TILE
    hb = (src_all >= HALF).astype(np.int64)
    gid = (core * NT + tl) * 2 + hb
    order = np.argsort(gid, kind="stable")
    gsrc = src_all[order]
    gdst = dst_all[order]
    gid_s = gid[order]

    counts = np.bincount(gid, minlength=NCORES * NT * 2).reshape(NCORES, NT, 2)
    # chunks per (tile, half): max over cores so the instruction stream is shared
    KA = _ceil_div(counts[:, :, 0].max(axis=0), TILE)  # [NT]
    KB = _ceil_div(counts[:, :, 1].max(axis=0), TILE)  # [NT]
    K = KA + KB

    # supertile structure ---------------------------------------------------
    st_tiles = [list(range(s * ST_TILES, min((s + 1) * ST_TILES, NT))) for s in range(NST)]

    # global chunk ids: per supertile: A-chunks tile-major, then B-chunks
    gbaseA = np.zeros(NT, dtype=np.int64)
    gbaseB = np.zeros(NT, dtype=np.int64)
    # position of chunk within its supertile's msg buffer
    lbaseA = np.zeros(NT, dtype=np.int64)
    lbaseB = np.zeros(NT, dtype=np.int64)
    st_of_tile = np.zeros(NT, dtype=np.int64)
    st_chunk_off = np.zeros(NST, dtype=np.int64)  # global chunk id of supertile start
    st_nchunks = np.zeros(NST, dtype=np.int64)
    g = 0
    for s, tiles in enumerate(st_tiles):
        st_chunk_off[s] = g
        off = 0
        for t in tiles:
            st_of_tile[t] = s
            gbaseA[t] = g
            lbaseA[t] = off
            g += KA[t]
            off += KA[t]
        for t in tiles:
            gbaseB[t] = g
            lbaseB[t] = off
            g += KB[t]
            off += KB[t]
        st_nchunks[s] = off
    TOTCH = g

    # gather calls are per (tile, half) group; idx columns follow global
    # chunk order, so chunk g owns idx columns [g*8, (g+1)*8).
    TOTIDX16 = TOTCH * (TILE // 16)  # idx array free-dim length (int16 cols)

    # ---------------------------------------------------------------- per-msg
    # position within (core, tile, half) group
    gstart = np.zeros(NCORES * NT * 2 + 1, dtype=np.int64)
    np.cumsum(np.bincount(gid_s, minlength=NCORES * NT * 2), out=gstart[1:])
    pos = np.arange(M, dtype=np.int64) - gstart[gid_s]

    m_core = gid_s // (NT * 2)
    m_tile = (gid_s // 2) % NT
    m_half = gid_s % 2
    m_chunk_in_group = pos // TILE
    m_part = pos % TILE
    m_gchunk = np.where(m_half == 0, gbaseA[m_tile], gbaseB[m_tile]) + m_chunk_in_group
    m_dstloc = gdst - (m_core * NPC + m_tile * TILE)
    m_idx16 = np.where(m_half == 0, gsrc, gsrc - HALF).astype(np.int16)
    m_col = m_gchunk * (TILE // 16) + (pos % TILE) // 16
    m_row16 = pos % 16

    # ---------------------------------------------------------------- arrays
    plan = Plan()
    plan.N, plan.NPC, plan.NT, plan.NTP, plan.NST = N, NPC, NT, NTP, NST
    plan.st_tiles = st_tiles
    plan.KA, plan.KB, plan.K = KA, KB, K
    plan.gbaseA, plan.gbaseB = gbaseA, gbaseB
    plan.st_chunk_off, plan.st_nchunks = st_chunk_off, st_nchunks
    plan.lbaseA, plan.lbaseB = lbaseA, lbaseB
    plan.TOTCH, plan.TOTIDX16 = TOTCH, TOTIDX16
    plan.dinv = dinv
    plan.iota = np.broadcast_to(
        np.arange(TILE, dtype=ml_dtypes.bfloat16)[None, :], (TILE, TILE)
    ).copy()

    # full dinv*x table for layer-0 gathers (replicated input; cols 96:128 pad)
    xd = x * dinv[:, None]
    xtab = np.zeros((N, TILE), dtype=ml_dtypes.bfloat16)
    xtab[:, :D] = xd.astype(ml_dtypes.bfloat16)
    plan.xtab = xtab

    per_core = []
    for c in range(NCORES):
        sel = m_core == c
        # dst-column per (chunk, slot) for on-device one-hot S generation:
        # S[p, g*128+c] = (dstv[p, g] == c); padded slots get -1 (no match)
        dstv = np.full((TILE, TOTCH), -1.0, dtype=ml_dtypes.bfloat16)
        dstv[m_part[sel], m_gchunk[sel]] = m_dstloc[sel].astype(
            ml_dtypes.bfloat16
        )
        # idx [128, TOTIDX16] int16 (wrapped by 16, replicated across 8 groups)
        # padding entries stay 0 (gather row 0; S column is zero there).
        # NOTE: negative-index trimming wedges the device with this ring
        # setup (decode-side ring-space accounting uses the untrimmed count)
        idx16 = np.zeros((16, TOTIDX16), dtype=np.int16)
        idx16[m_row16[sel], m_col[sel]] = m_idx16[sel]
        idx = np.tile(idx16, (8, 1))
        # xTd [96, NTP] bf16: own rows of dinv*x, transposed (self-loop add)
        xTd = np.zeros((D, NTP), dtype=ml_dtypes.bfloat16)
        xTd[:, :NPC] = xd[c * NPC : (c + 1) * NPC].T.astype(ml_dtypes.bfloat16)
        # dinvT replicated [96, NTP]
        dinvT = np.ones((D, NTP), dtype=np.float32)
        dinvT[:, :NPC] = dinv[c * NPC : (c + 1) * NPC][None, :]
        # dinv per own row, tile-column layout [128, NT]
        downv = np.ones((TILE, NT), dtype=np.float32)
        dv = dinv[c * NPC : (c + 1) * NPC]
        dvp = np.zeros(NTP, dtype=np.float32)
        dvp[:NPC] = dv
        downv[:, :] = dvp.reshape(NT, TILE).T
        per_core.append(dict(dstv=dstv, idx=idx, xTd=xTd, dinvT=dinvT, dinvown=downv))
    plan.per_core = per_core
    return plan


# ---------------------------------------------------------------------------
# Bass program builder
# ---------------------------------------------------------------------------


def _build(plan, repeats=1, skip=frozenset()):
    import concourse.bass as bass
    import concourse.bacc as bacc
    import concourse.mybir as mybir
    import concourse.tile as tile

    f32 = mybir.dt.float32
    bf16 = mybir.dt.bfloat16
    fp8 = mybir.dt.float8e4
    i16 = mybir.dt.int16
    AF = mybir.ActivationFunctionType
    ALU = mybir.AluOpType

    N, NPC, NT, NTP, NST = plan.N, plan.NPC, plan.NT, plan.NTP, plan.NST
    TOTCH, TOTIDX16 = plan.TOTCH, plan.TOTIDX16
    KA, KB = plan.KA, plan.KB

    nc = bacc.Bacc(
        None,
        target_bir_lowering=False,
        num_swdge_queues=NUM_QUEUES,
        dynamic_dma_scratch_size=DMA_SCRATCH,
    )

    xtab_p = nc.declare_dram_parameter("xtab", [N, TILE], bf16, isOutput=False)
    xTd_p = nc.declare_dram_parameter("xTd", [D, NTP], bf16, isOutput=False)
    idx_p = nc.declare_dram_parameter("idx", [TILE, TOTIDX16], i16, isOutput=False)
    dstv_p = nc.declare_dram_parameter("dstv", [TILE, TOTCH], bf16, isOutput=False)
    iota_p = nc.declare_dram_parameter("iota", [TILE, TILE], bf16, isOutput=False)
    dinvT_p = nc.declare_dram_parameter("dinvT", [D, NTP], f32, isOutput=False)
    dinvown_p = nc.declare_dram_parameter("dinvown", [TILE, NT], f32, isOutput=False)
    biasT_p = nc.declare_dram_parameter("biasT", [D, N_LAYERS], f32, isOutput=False)
    brep_p = nc.declare_dram_parameter("brep", [TILE, 4], f32, isOutput=False)
    w_p = nc.declare_dram_parameter("W", [D, N_LAYERS * D], f32, isOutput=False)
    wl_p = nc.declare_dram_parameter("Wl", [D, 4], f32, isOutput=False)
    out_p = nc.declare_dram_parameter("out", [NPC, 4], f32, isOutput=True)

    replica_groups = [list(range(NCORES))]

    with tile.TileContext(nc) as tc:
        with (
            tc.tile_pool(name="persist", bufs=1) as persist,
            tc.tile_pool(name="hrelu", bufs=2) as hrelu_pool,
            tc.tile_pool(name="hdinv", bufs=2) as hdinv_pool,
            tc.tile_pool(name="msg", bufs=MSG_BUFS) as msg_pool,
            tc.tile_pool(name="spool", bufs=2) as s_pool,
            tc.tile_pool(name="tmp", bufs=3) as tmp_pool,
            tc.tile_pool(name="stage", bufs=3) as stage_pool,
            tc.tile_pool(name="small", bufs=3) as small_pool,
            tc.tile_pool(name="psA", bufs=6, space="PSUM") as psA_pool,
            tc.tile_pool(name="ps3", bufs=2, space="PSUM") as ps3_pool,
            tc.tile_pool(name="dram", bufs=2, space="DRAM") as dram_pool,
        ):
            # ------------------------------------------------- persistent loads
            xTd_sb = persist.tile([D, NTP], bf16, tag="xTd")
            nc.sync.dma_start(xTd_sb[:], xTd_p[:])
            idx_sb = persist.tile([TILE, TOTIDX16], i16, tag="idx")
            nc.sync.dma_start(idx_sb[:], idx_p[:])
            dinvT_sb = persist.tile([D, NTP], f32, tag="dinvT")
            nc.sync.dma_start(dinvT_sb[:], dinvT_p[:])
            dinvown_sb = persist.tile([TILE, NT], f32, tag="dinvown")
            nc.sync.dma_start(dinvown_sb[:], dinvown_p[:])
            biasT_sb = persist.tile([D, N_LAYERS], f32, tag="biasT")
            nc.sync.dma_start(biasT_sb[:], biasT_p[:])
            brep_sb = persist.tile([TILE, 4], f32, tag="brep")
            nc.sync.dma_start(brep_sb[:], brep_p[:])
            dstv_sb = persist.tile([TILE, TOTCH], bf16, tag="dstv")
            nc.sync.dma_start(dstv_sb[:], dstv_p[:])
            iota_sb = persist.tile([TILE, TILE], bf16, tag="iota")
            nc.sync.dma_start(iota_sb[:], iota_p[:])
            w_sb = persist.tile([D, N_LAYERS * D], bf16, tag="W")
            nc.gpsimd.dma_start(w_sb[:], w_p[:])
            wl_sb = persist.tile([D, 4], bf16, tag="Wl")
            nc.gpsimd.dma_start(wl_sb[:], wl_p[:])

            def transform_stage(t, layer, src_T, ag_in):
                """h_t tile = (src_T[:, t] @ W[layer]) * dinv, staged to ag_in."""
                w = min(TILE, NPC - t * TILE)
                ps3 = ps3_pool.tile([TILE, D], f32, tag="ps3")
                nc.tensor.matmul(
                    ps3[:w],
                    src_T[:, t * TILE : t * TILE + w],
                    w_sb[:, layer * D : (layer + 1) * D],
                    start=True,
                    stop=True,
                )
                # pad cols 96:128 stay garbage: gathered but never read
                # (chunk matmuls consume msg3[:, j, 0:D] only)
                st = stage_pool.tile([TILE, TILE], bf16, tag="stage")
                nc.scalar.activation(
                    st[:w, 0:D], ps3[:w], AF.Copy, scale=dinvown_sb[:w, t : t + 1]
                )
                nc.sync.dma_start(ag_in[t * TILE : t * TILE + w, :], st[:w])

            def allgather(ag_in, ag_out):
                if "ag" in skip:
                    for r in range(NCORES):
                        nc.sync.dma_start(ag_out[r * NPC : (r + 1) * NPC, :], ag_in[:])
                else:
                    nc.gpsimd.collective_compute(
                        "AllGather",
                        ALU.bypass,
                        replica_groups=replica_groups,
                        ins=[ag_in[:]],
                        outs=[ag_out[:]],
                    )

            def classifier(t, src_T):
                w = min(TILE, NPC - t * TILE)
                psf = ps3_pool.tile([TILE, D], f32, tag="ps3")
                nc.tensor.matmul(
                    psf[:w, 0:4],
                    src_T[:, t * TILE : t * TILE + w],
                    wl_sb[:],
                    start=True,
                    stop=True,
                )
                xb = small_pool.tile([TILE, 4], f32, tag="xb")
                nc.vector.tensor_tensor(xb[:w], psf[:w, 0:4], brep_sb[:w], ALU.add)
                negm = small_pool.tile([TILE, 1], f32, tag="negm")
                nc.vector.tensor_reduce(
                    negm[:w], xb[:w], mybir.AxisListType.X, ALU.max, negate=True
                )
                ex = small_pool.tile([TILE, 4], f32, tag="ex")
                sumexp = small_pool.tile([TILE, 1], f32, tag="sumexp")
                nc.scalar.activation(
                    ex[:w], xb[:w], AF.Exp, bias=negm[:w], accum_out=sumexp[:w]
                )
                lse = small_pool.tile([TILE, 1], f32, tag="lse")
                nc.scalar.activation(lse[:w], sumexp[:w], AF.Ln)
                shift = small_pool.tile([TILE, 1], f32, tag="shift")
                nc.vector.tensor_sub(shift[:w], negm[:w], lse[:w])
                outt = small_pool.tile([TILE, 4], f32, tag="outt")
                nc.vector.tensor_scalar_add(outt[:w], xb[:w], shift[:w])
                nc.sync.dma_start(out_p[t * TILE : t * TILE + w, :], outt[:w])

            s_live = "sdma" not in skip

            def gen_S(s):
                """One-hot S for supertile s, on DVE from dstv/iota broadcasts.
                S is layer-independent; callers issue this one supertile ahead
                so the in-order DVE queue never stalls matmuls on it."""
                nch = int(plan.st_nchunks[s])
                goff = int(plan.st_chunk_off[s])
                s_sb = s_pool.tile([TILE, nch * TILE], fp8, tag="spool")
                s3 = s_sb[:].rearrange("p (c e) -> p c e", e=TILE)
                ngen = nch if s_live else 1
                nc.vector.tensor_tensor(
                    s3[:, 0:ngen, :],
                    dstv_sb[:, goff : goff + ngen]
                    .unsqueeze(-1)
                    .broadcast_to([TILE, ngen, TILE]),
                    iota_sb[:].unsqueeze(1).broadcast_to([TILE, ngen, TILE]),
                    ALU.is_equal,
                )
                return s_sb

            for rep in range(repeats):
              # layer 0 gathers straight from the host-prepared dinv*x table;
              # W0 is applied after aggregation (linearity), so there is no
              # transform/stage/AllGather prologue at all.
              ag_out = None
              cur_D = None
              s_next = gen_S(0) if HOIST_S else None
              for layer in range(N_LAYERS):
                last = layer == N_LAYERS - 1
                new_T = hrelu_pool.tile([D, NTP], bf16, tag="hrelu")
                if not last:
                    new_D = hdinv_pool.tile([D, NTP], bf16, tag="hdinv")
                    ag_in2 = dram_pool.tile([NPC, TILE], bf16, tag="ag_in")
                    ag_out2 = dram_pool.tile(
                        [N, TILE],
                        bf16,
                        tag="ag_out",
                        addr_space="Local" if "ag" in skip else "Shared",
                    )
                gq = [0]
                for s, tiles in enumerate(plan.st_tiles):
                    nch = int(plan.st_nchunks[s])
                    goff = int(plan.st_chunk_off[s])
                    msg = msg_pool.tile([TILE, nch * TILE], bf16, tag="msg")
                    msg3 = msg[:].rearrange("p (c e) -> p c e", e=TILE)
                    s_sb = s_next if HOIST_S else gen_S(s)
                    g_live = "gather" not in skip
                    # pre-generate the next supertile's S (same for all layers)
                    if HOIST_S:
                        if s + 1 < plan.NST:
                            s_next = gen_S(s + 1)
                        elif not last or rep + 1 < repeats:
                            s_next = gen_S(0)
                    # gather calls split to <=1024 indices per call (SWDGE
                    # ring capacity)
                    GMAX = GATHER_CALL_IDX
                    table = xtab_p if layer == 0 else ag_out
                    if not g_live:
                        nc.gpsimd.dma_gather(
                            msg3[:, 0:1, :], table[:],
                            idx_sb[:, goff * 8 : goff * 8 + 8],
                            num_idxs=TILE, num_idxs_reg=TILE, elem_size=TILE,
                        )
                    for t in tiles:
                        for h in range(2):
                            k = int((KA if h == 0 else KB)[t])
                            if k == 0 or not g_live:
                                continue
                            n = k * TILE
                            gchunk0 = int((plan.gbaseA if h == 0 else plan.gbaseB)[t])
                            lbase = int((plan.lbaseA if h == 0 else plan.lbaseB)[t])
                            coloff = gchunk0 * 8
                            in_ap = table[:] if h == 0 else table[HALF:N, :]
                            for c0 in range(0, n, GMAX):
                                nn = min(GMAX, n - c0)
                                out_ap = msg3[
                                    :,
                                    lbase + c0 // TILE : lbase + (c0 + nn) // TILE,
                                    :,
                                ]
                                nc.gpsimd.dma_gather(
                                    out_ap,
                                    in_ap,
                                    idx_sb[
                                        :, coloff + c0 // 16 : coloff + (c0 + nn) // 16
                                    ],
                                    num_idxs=nn,
                                    num_idxs_reg=nn,
                                    elem_size=TILE,
                                    queue_num=gq[0] % NUM_QUEUES,
                                    single_packet=SINGLE_PACKET,
                                )
                                gq[0] += 1
                    # per-tile accumulation, tile-major chunk order; the
                    # self-loop term dinv_i^2 (h W)_i enters the same PSUM as
                    # a W^T @ (h*dinv) matmul (contraction over features)
                    for t in tiles:
                        kA, kB = int(KA[t]), int(KB[t])
                        lA = int(plan.gbaseA[t] - goff)
                        lB = int(plan.gbaseB[t] - goff)
                        locs = [lA + j for j in range(kA)] + [lB + j for j in range(kB)]
                        if "mm" in skip:
                            locs = locs[:1]
                        psA = psA_pool.tile([D, TILE], f32, tag="psA")
                        if layer > 0:
                            nc.tensor.matmul(
                                psA[:],
                                w_sb[:, layer * D : (layer + 1) * D],
                                cur_D[:, t * TILE : (t + 1) * TILE],
                                start=True,
                                stop=(len(locs) == 0),
                            )
                        for ji, j in enumerate(locs):
                            nc.tensor.matmul(
                                psA[:],
                                msg3[:, j if g_live else 0, 0:D],
                                s_sb[:, (j if s_live else 0) * TILE : ((j if s_live else 0) + 1) * TILE],
                                start=(layer == 0 and ji == 0),
                                stop=(ji == len(locs) - 1),
                            )
                        if layer == 0:
                            # agg = sum dinv_src*x_src (+ self-loop dinv_i*x_i),
                            # then apply W0 (commuted past the aggregation)
                            agg = tmp_pool.tile([D, TILE], bf16, tag="agg")
                            nc.vector.tensor_tensor(
                                agg[:],
                                psA[:],
                                xTd_sb[:, t * TILE : (t + 1) * TILE],
                                ALU.add,
                            )
                            psA = psA_pool.tile([D, TILE], f32, tag="psA")
                            nc.tensor.matmul(
                                psA[:],
                                w_sb[:, 0:D],
                                agg[:],
                                start=True,
                                stop=True,
                            )
                        tmp = tmp_pool.tile([D, TILE], f32, tag="tmp")
                        nc.vector.tensor_tensor(
                            tmp[:], psA[:], dinvT_sb[:, t * TILE : (t + 1) * TILE], ALU.mult
                        )
                        nc.scalar.activation(
                            new_T[:, t * TILE : (t + 1) * TILE],
                            tmp[:],
                            AF.Relu,
                            bias=biasT_sb[:, layer : layer + 1],
                        )
                        if not last:
                            nc.vector.tensor_tensor(
                                new_D[:, t * TILE : (t + 1) * TILE],
                                new_T[:, t * TILE : (t + 1) * TILE],
                                dinvT_sb[:, t * TILE : (t + 1) * TILE],
                                ALU.mult,
                            )
                            transform_stage(t, layer + 1, new_T, ag_in2)
                        else:
                            classifier(t, new_T)
                if not last:
                    allgather(ag_in2, ag_out2)
                    ag_out = ag_out2
                    cur_T, cur_D = new_T, new_D
                else:
                    cur_T = new_T

    nc.compile()
    return nc


# ---------------------------------------------------------------------------
# in_maps assembly
# ---------------------------------------------------------------------------


def _in_maps(plan, W0, b0, W1, b1, W2, b2, W3, b3, Wl, bl):
    Ws = np.concatenate(
        [np.asarray(w, np.float32) for w in (W0, W1, W2, W3)], axis=1
    )  # [96, 4*96]
    biasT = np.stack(
        [np.asarray(b, np.float32) for b in (b0, b1, b2, b3)], axis=1
    )  # [96, 4]
    brep = np.tile(np.asarray(bl, np.float32)[None, :], (TILE, 1))  # [128, 4]
    wl = np.asarray(Wl, np.float32)
    maps = []
    for c in range(NCORES):
        pc = plan.per_core[c]
        maps.append(
            {
                "xtab": plan.xtab,
                "xTd": pc["xTd"],
                "idx": pc["idx"],
                "dstv": pc["dstv"],
                "iota": plan.iota,
                "dinvT": pc["dinvT"],
                "dinvown": pc["dinvown"],
                "biasT": biasT,
                "brep": brep,
                "W": Ws,
                "Wl": wl,
                "out": np.zeros((plan.NPC, 4), np.float32),
            }
        )
    return maps


# ---------------------------------------------------------------------------
# public entry point
# ---------------------------------------------------------------------------

_CACHE = {}


def _get_compiled(plan):
    return _build(plan)


def kernel(x, edge_index, W0, b0, W1, b1, W2, b2, W3, b3, Wl, bl):
    from concourse.bass_utils import run_bass_kernel_spmd

    x = np.asarray(x, np.float32)
    edge_index = np.asarray(edge_index, np.int64)
    plan = _prep(x, edge_index)
    nc = _get_compiled(plan)
    in_maps = _in_maps(plan, W0, b0, W1, b1, W2, b2, W3, b3, Wl, bl)
    res = run_bass_kernel_spmd(nc, in_maps, core_ids=list(range(NCORES)))
    out = np.concatenate([res.results[c]["out"] for c in range(NCORES)], axis=0)
    return out.astype(np.float32)



# revision 29
# speedup vs baseline: 1.1969x; 1.0229x over previous
"""GCN (4-layer, PyG GCNConv-style) Trainium2 Bass kernel, SPMD over 8 NeuronCores.

Strategy
--------
Nodes are sharded in contiguous blocks across 8 cores; edges are partitioned by
destination node.  Pipeline per layer (supertile = 4 dst tiles):
  * gather: per-edge source rows fetched with dma_gather (256B descriptors,
    4 SWDGE queues).  Layer 0 gathers straight from a host-prepared dinv*x
    table (a replicated input) and W0 is applied AFTER aggregation
    (linearity commute) - no transform/stage/AllGather prologue exists.
  * scatter-add: one-hot S chunks are GENERATED ON DEVICE (one DVE is_equal
    per supertile from a [128, TOTCH] bf16 dst-column table against an iota
    row, hoisted one supertile ahead so the in-order DVE queue never blocks
    matmuls); S contracts gathered message chunks on the PE into PSUM.  The
    self-loop term enters the same PSUM as a W^T @ (h*dinv) matmul.
  * epilogue: dinv[dst] scale + bias + relu; then the NEXT layer's transform
    (h @ W * dinv, PE) for the same tiles is staged immediately to the
    AllGather input so the shared-output AllGather (~45-50us on-chip) fires
    right after the last supertile; layer-3 tiles run the classifier
    (log_softmax) instead.
Messages split into halves by source row (< 32768 / >= 32768) with a re-based
source view for the second half (dma_gather int16 index limit).

Perf notes (measured, axon-tunneled TRN2, interleaved repeat-delta timing):
  * dma_gather steady state ~1.4-1.8ns per 256B msg (HBM random-read bound,
    NOT desc-gen bound); kernel achieves ~2.0ns/slot including contention.
  * >1024-idx calls do work with dynamic_dma_scratch_size scaled up, but are
    perf-neutral; single_packet and queue count >=4 likewise.
  * negative-index trimming (trailing -1 idxs) WEDGES the device unless
    num_idxs_reg carries the per-core post-trim count (decode-side ring
    accounting reads the register, gen reads the trimmed static count).
  * staged-table pad columns 96:128 are never read by consumers - garbage OK.
Measured ~1.55-1.8 ms per forward pass, rel err 6e-4 (was 2.2-2.7 ms).
"""

import math
import os
import sys

import numpy as np

sys.path.insert(0, "/opt/trn_rl_repo")

import ml_dtypes  # noqa: E402

NCORES = 8
TILE = 128
D = 96
HALF = 32768  # int16-addressable row limit for dma_gather indices
ST_TILES = 4  # tiles per supertile (one gather call pair per supertile)
N_LAYERS = 4
SINGLE_PACKET = False
MSG_BUFS = 3
GATHER_CALL_IDX = 1024  # <=1024: SWDGE ring capacity
NUM_QUEUES = 4
DMA_SCRATCH = 16384  # descriptor carveout: ring capacity = this/16 descs
HOIST_S = True  # pre-generate next supertile S one step ahead on DVE



def _ceil_div(a, b):
    return -(-a // b)


# ---------------------------------------------------------------------------
# Host-side preprocessing
# ---------------------------------------------------------------------------


class Plan:
    """Shared (core-independent) structure + per-core data arrays."""

    pass


def _prep(x, edge_index):
    """Build the shared chunk structure and per-core input arrays."""
    x = np.asarray(x, dtype=np.float32)
    edge_index = np.asarray(edge_index, dtype=np.int64)
    N, d_in = x.shape
    assert d_in == D
    NPC = N // NCORES
    assert NPC * NCORES == N
    NT = _ceil_div(NPC, TILE)
    NTP = NT * TILE
    NST = _ceil_div(NT, ST_TILES)

    # self-loop messages are computed on-device as a W^T @ (h*dinv) matmul
    # accumulated into the same PSUM as the edge chunks, so the gather
    # structure only carries the real edges.
    src_all = edge_index[0]
    dst_all = edge_index[1]
    M = src_all.shape[0]

    deg = (np.bincount(dst_all, minlength=N) + 1).astype(np.float32)
    dinv = (1.0 / np.sqrt(deg)).astype(np.float32)

    core = dst_all // NPC
    tl = (dst_all % NPC) // TILE
    SPL = 4096  # per-core local-row split: T1 = 8*4096 rows (int16-exact)
    s_core = src_all // NPC
    s_loc = src_all % NPC
    hb = (s_loc >= SPL).astype(np.int64)
    gid = (core * NT + tl) * 2 + hb
    order = np.argsort(gid, kind="stable")
    gsrc = src_all[order]
    gdst = dst_all[order]
    gid_s = gid[order]

    counts = np.bincount(gid, minlength=NCORES * NT * 2).reshape(NCORES, NT, 2)
    # chunks per (tile, half): max over cores so the instruction stream is shared
    KA = _ceil_div(counts[:, :, 0].max(axis=0), TILE)  # [NT]
    KB = _ceil_div(counts[:, :, 1].max(axis=0), TILE)  # [NT]
    K = KA + KB

    # supertile structure ---------------------------------------------------
    st_tiles = [list(range(s * ST_TILES, min((s + 1) * ST_TILES, NT))) for s in range(NST)]

    # global chunk ids: per supertile: A-chunks tile-major, then B-chunks
    gbaseA = np.zeros(NT, dtype=np.int64)
    gbaseB = np.zeros(NT, dtype=np.int64)
    # position of chunk within its supertile's msg buffer
    lbaseA = np.zeros(NT, dtype=np.int64)
    lbaseB = np.zeros(NT, dtype=np.int64)
    st_of_tile = np.zeros(NT, dtype=np.int64)
    st_chunk_off = np.zeros(NST, dtype=np.int64)  # global chunk id of supertile start
    st_nchunks = np.zeros(NST, dtype=np.int64)
    g = 0
    for s, tiles in enumerate(st_tiles):
        st_chunk_off[s] = g
        off = 0
        for t in tiles:
            st_of_tile[t] = s
            gbaseA[t] = g
            lbaseA[t] = off
            g += KA[t]
            off += KA[t]
        for t in tiles:
            gbaseB[t] = g
            lbaseB[t] = off
            g += KB[t]
            off += KB[t]
        st_nchunks[s] = off
    TOTCH = g

    # gather calls are per (tile, half) group; idx columns follow global
    # chunk order, so chunk g owns idx columns [g*8, (g+1)*8).
    TOTIDX16 = TOTCH * (TILE // 16)  # idx array free-dim length (int16 cols)

    # ---------------------------------------------------------------- per-msg
    # position within (core, tile, half) group
    gstart = np.zeros(NCORES * NT * 2 + 1, dtype=np.int64)
    np.cumsum(np.bincount(gid_s, minlength=NCORES * NT * 2), out=gstart[1:])
    pos = np.arange(M, dtype=np.int64) - gstart[gid_s]

    m_core = gid_s // (NT * 2)
    m_tile = (gid_s // 2) % NT
    m_half = gid_s % 2
    m_chunk_in_group = pos // TILE
    m_part = pos % TILE
    m_gchunk = np.where(m_half == 0, gbaseA[m_tile], gbaseB[m_tile]) + m_chunk_in_group
    m_dstloc = gdst - (m_core * NPC + m_tile * TILE)
    g_core = gsrc // NPC
    g_loc = gsrc % NPC
    m_idx16 = np.where(
        m_half == 0, g_core * SPL + g_loc, g_core * (NPC - SPL) + g_loc - SPL
    ).astype(np.int16)
    m_col = m_gchunk * (TILE // 16) + (pos % TILE) // 16
    m_row16 = pos % 16

    # ---------------------------------------------------------------- arrays
    plan = Plan()
    plan.N, plan.NPC, plan.NT, plan.NTP, plan.NST = N, NPC, NT, NTP, NST
    plan.st_tiles = st_tiles
    plan.KA, plan.KB, plan.K = KA, KB, K
    plan.gbaseA, plan.gbaseB = gbaseA, gbaseB
    plan.st_chunk_off, plan.st_nchunks = st_chunk_off, st_nchunks
    plan.lbaseA, plan.lbaseB = lbaseA, lbaseB
    plan.TOTCH, plan.TOTIDX16 = TOTCH, TOTIDX16
    plan.dinv = dinv
    plan.iota = np.broadcast_to(
        np.arange(TILE, dtype=ml_dtypes.bfloat16)[None, :], (TILE, TILE)
    ).copy()

    # full dinv*x tables for layer-0 gathers, laid out in the same
    # T1/T2 local-row-split convention as the staged tables
    xd = (x * dinv[:, None]).astype(ml_dtypes.bfloat16)
    xd4 = xd.reshape(NCORES, NPC, D)
    xtab1 = np.zeros((NCORES * SPL, TILE), dtype=ml_dtypes.bfloat16)
    xtab1[:, :D] = xd4[:, :SPL].reshape(-1, D)
    xtab2 = np.zeros((NCORES * (NPC - SPL), TILE), dtype=ml_dtypes.bfloat16)
    xtab2[:, :D] = xd4[:, SPL:].reshape(-1, D)
    plan.xtab1, plan.xtab2 = xtab1, xtab2
    plan.SPL = SPL

    per_core = []
    for c in range(NCORES):
        sel = m_core == c
        # dst-column per (chunk, slot) for on-device one-hot S generation:
        # S[p, g*128+c] = (dstv[p, g] == c); padded slots get -1 (no match)
        dstv = np.full((TILE, TOTCH), -1.0, dtype=ml_dtypes.bfloat16)
        dstv[m_part[sel], m_gchunk[sel]] = m_dstloc[sel].astype(
            ml_dtypes.bfloat16
        )
        # idx [128, TOTIDX16] int16 (wrapped by 16, replicated across 8 groups)
        # padding entries stay 0 (gather row 0; S column is zero there).
        # NOTE: negative-index trimming wedges the device with this ring
        # setup (decode-side ring-space accounting uses the untrimmed count)
        idx16 = np.zeros((16, TOTIDX16), dtype=np.int16)
        idx16[m_row16[sel], m_col[sel]] = m_idx16[sel]
        idx = np.tile(idx16, (8, 1))
        # xTd [96, NTP] bf16: own rows of dinv*x, transposed (self-loop add)
        xTd = np.zeros((D, NTP), dtype=ml_dtypes.bfloat16)
        xTd[:, :NPC] = xd[c * NPC : (c + 1) * NPC].T.astype(ml_dtypes.bfloat16)
        # dinvT replicated [96, NTP]
        dinvT = np.ones((D, NTP), dtype=np.float32)
        dinvT[:, :NPC] = dinv[c * NPC : (c + 1) * NPC][None, :]
        # dinv per own row, tile-column layout [128, NT]
        downv = np.ones((TILE, NT), dtype=np.float32)
        dv = dinv[c * NPC : (c + 1) * NPC]
        dvp = np.zeros(NTP, dtype=np.float32)
        dvp[:NPC] = dv
        downv[:, :] = dvp.reshape(NT, TILE).T
        per_core.append(dict(dstv=dstv, idx=idx, xTd=xTd, dinvT=dinvT, dinvown=downv))
    plan.per_core = per_core
    return plan


# ---------------------------------------------------------------------------
# Bass program builder
# ---------------------------------------------------------------------------


def _build(plan, repeats=1, skip=frozenset()):
    import concourse.bass as bass
    import concourse.bacc as bacc
    import concourse.mybir as mybir
    import concourse.tile as tile

    f32 = mybir.dt.float32
    bf16 = mybir.dt.bfloat16
    fp8 = mybir.dt.float8e4
    i16 = mybir.dt.int16
    AF = mybir.ActivationFunctionType
    ALU = mybir.AluOpType

    N, NPC, NT, NTP, NST = plan.N, plan.NPC, plan.NT, plan.NTP, plan.NST
    TOTCH, TOTIDX16 = plan.TOTCH, plan.TOTIDX16
    KA, KB = plan.KA, plan.KB

    nc = bacc.Bacc(
        None,
        target_bir_lowering=False,
        num_swdge_queues=NUM_QUEUES,
        dynamic_dma_scratch_size=DMA_SCRATCH,
    )

    SPL = plan.SPL
    xtab1_p = nc.declare_dram_parameter(
        "xtab1", [NCORES * SPL, TILE], bf16, isOutput=False
    )
    xtab2_p = nc.declare_dram_parameter(
        "xtab2", [NCORES * (NPC - SPL), TILE], bf16, isOutput=False
    )
    xTd_p = nc.declare_dram_parameter("xTd", [D, NTP], bf16, isOutput=False)
    idx_p = nc.declare_dram_parameter("idx", [TILE, TOTIDX16], i16, isOutput=False)
    dstv_p = nc.declare_dram_parameter("dstv", [TILE, TOTCH], bf16, isOutput=False)
    iota_p = nc.declare_dram_parameter("iota", [TILE, TILE], bf16, isOutput=False)
    dinvT_p = nc.declare_dram_parameter("dinvT", [D, NTP], f32, isOutput=False)
    dinvown_p = nc.declare_dram_parameter("dinvown", [TILE, NT], f32, isOutput=False)
    biasT_p = nc.declare_dram_parameter("biasT", [D, N_LAYERS], f32, isOutput=False)
    brep_p = nc.declare_dram_parameter("brep", [TILE, 4], f32, isOutput=False)
    w_p = nc.declare_dram_parameter("W", [D, N_LAYERS * D], f32, isOutput=False)
    wl_p = nc.declare_dram_parameter("Wl", [D, 4], f32, isOutput=False)
    out_p = nc.declare_dram_parameter("out", [NPC, 4], f32, isOutput=True)

    replica_groups = [list(range(NCORES))]

    with tile.TileContext(nc) as tc:
        with (
            tc.tile_pool(name="persist", bufs=1) as persist,
            tc.tile_pool(name="hrelu", bufs=2) as hrelu_pool,
            tc.tile_pool(name="hdinv", bufs=2) as hdinv_pool,
            tc.tile_pool(name="msg", bufs=MSG_BUFS) as msg_pool,
            tc.tile_pool(name="spool", bufs=2) as s_pool,
            tc.tile_pool(name="tmp", bufs=3) as tmp_pool,
            tc.tile_pool(name="stage", bufs=3) as stage_pool,
            tc.tile_pool(name="small", bufs=3) as small_pool,
            tc.tile_pool(name="psA", bufs=6, space="PSUM") as psA_pool,
            tc.tile_pool(name="ps3", bufs=2, space="PSUM") as ps3_pool,
            tc.tile_pool(name="dram", bufs=2, space="DRAM") as dram_pool,
        ):
            # ------------------------------------------------- persistent loads
            xTd_sb = persist.tile([D, NTP], bf16, tag="xTd")
            nc.sync.dma_start(xTd_sb[:], xTd_p[:])
            idx_sb = persist.tile([TILE, TOTIDX16], i16, tag="idx")
            nc.sync.dma_start(idx_sb[:], idx_p[:])
            dinvT_sb = persist.tile([D, NTP], f32, tag="dinvT")
            nc.sync.dma_start(dinvT_sb[:], dinvT_p[:])
            dinvown_sb = persist.tile([TILE, NT], f32, tag="dinvown")
            nc.sync.dma_start(dinvown_sb[:], dinvown_p[:])
            biasT_sb = persist.tile([D, N_LAYERS], f32, tag="biasT")
            nc.sync.dma_start(biasT_sb[:], biasT_p[:])
            brep_sb = persist.tile([TILE, 4], f32, tag="brep")
            nc.sync.dma_start(brep_sb[:], brep_p[:])
            dstv_sb = persist.tile([TILE, TOTCH], bf16, tag="dstv")
            nc.sync.dma_start(dstv_sb[:], dstv_p[:])
            iota_sb = persist.tile([TILE, TILE], bf16, tag="iota")
            nc.sync.dma_start(iota_sb[:], iota_p[:])
            w_sb = persist.tile([D, N_LAYERS * D], bf16, tag="W")
            nc.gpsimd.dma_start(w_sb[:], w_p[:])
            wl_sb = persist.tile([D, 4], bf16, tag="Wl")
            nc.gpsimd.dma_start(wl_sb[:], wl_p[:])

            def transform_stage(t, layer, src_T, ag_in):
                """h_t tile = (src_T[:, t] @ W[layer]) * dinv, staged to ag_in."""
                w = min(TILE, NPC - t * TILE)
                ps3 = ps3_pool.tile([TILE, D], f32, tag="ps3")
                nc.tensor.matmul(
                    ps3[:w],
                    src_T[:, t * TILE : t * TILE + w],
                    w_sb[:, layer * D : (layer + 1) * D],
                    start=True,
                    stop=True,
                )
                # pad cols 96:128 stay garbage: gathered but never read
                # (chunk matmuls consume msg3[:, j, 0:D] only)
                st = stage_pool.tile([TILE, TILE], bf16, tag="stage")
                nc.scalar.activation(
                    st[:w, 0:D], ps3[:w], AF.Copy, scale=dinvown_sb[:w, t : t + 1]
                )
                nc.sync.dma_start(ag_in[t * TILE : t * TILE + w, :], st[:w])

            def allgather_part(ag_in, ag_out, lo, hi):
                if "ag" in skip:
                    w = hi - lo
                    for r in range(NCORES):
                        nc.sync.dma_start(
                            ag_out[r * w : (r + 1) * w, :], ag_in[lo:hi, :]
                        )
                else:
                    nc.gpsimd.collective_compute(
                        "AllGather",
                        ALU.bypass,
                        replica_groups=replica_groups,
                        ins=[ag_in[lo:hi, :]],
                        outs=[ag_out[:]],
                    )

            def classifier(t, src_T):
                w = min(TILE, NPC - t * TILE)
                psf = ps3_pool.tile([TILE, D], f32, tag="ps3")
                nc.tensor.matmul(
                    psf[:w, 0:4],
                    src_T[:, t * TILE : t * TILE + w],
                    wl_sb[:],
                    start=True,
                    stop=True,
                )
                xb = small_pool.tile([TILE, 4], f32, tag="xb")
                nc.vector.tensor_tensor(xb[:w], psf[:w, 0:4], brep_sb[:w], ALU.add)
                negm = small_pool.tile([TILE, 1], f32, tag="negm")
                nc.vector.tensor_reduce(
                    negm[:w], xb[:w], mybir.AxisListType.X, ALU.max, negate=True
                )
                ex = small_pool.tile([TILE, 4], f32, tag="ex")
                sumexp = small_pool.tile([TILE, 1], f32, tag="sumexp")
                nc.scalar.activation(
                    ex[:w], xb[:w], AF.Exp, bias=negm[:w], accum_out=sumexp[:w]
                )
                lse = small_pool.tile([TILE, 1], f32, tag="lse")
                nc.scalar.activation(lse[:w], sumexp[:w], AF.Ln)
                shift = small_pool.tile([TILE, 1], f32, tag="shift")
                nc.vector.tensor_sub(shift[:w], negm[:w], lse[:w])
                outt = small_pool.tile([TILE, 4], f32, tag="outt")
                nc.vector.tensor_scalar_add(outt[:w], xb[:w], shift[:w])
                nc.sync.dma_start(out_p[t * TILE : t * TILE + w, :], outt[:w])

            s_live = "sdma" not in skip

            def gen_S(s):
                """One-hot S for supertile s, on DVE from dstv/iota broadcasts.
                S is layer-independent; callers issue this one supertile ahead
                so the in-order DVE queue never stalls matmuls on it."""
                nch = int(plan.st_nchunks[s])
                goff = int(plan.st_chunk_off[s])
                s_sb = s_pool.tile([TILE, nch * TILE], fp8, tag="spool")
                s3 = s_sb[:].rearrange("p (c e) -> p c e", e=TILE)
                ngen = nch if s_live else 1
                nc.vector.tensor_tensor(
                    s3[:, 0:ngen, :],
                    dstv_sb[:, goff : goff + ngen]
                    .unsqueeze(-1)
                    .broadcast_to([TILE, ngen, TILE]),
                    iota_sb[:].unsqueeze(1).broadcast_to([TILE, ngen, TILE]),
                    ALU.is_equal,
                )
                return s_sb

            for rep in range(repeats):
              # layer 0 gathers straight from the host-prepared dinv*x table;
              # W0 is applied after aggregation (linearity), so there is no
              # transform/stage/AllGather prologue at all.
              tabA = tabB = None
              cur_D = None
              s_next = gen_S(0) if HOIST_S else None
              for layer in range(N_LAYERS):
                last = layer == N_LAYERS - 1
                new_T = hrelu_pool.tile([D, NTP], bf16, tag="hrelu")
                if not last:
                    new_D = hdinv_pool.tile([D, NTP], bf16, tag="hdinv")
                    ag_in2 = dram_pool.tile([NPC, TILE], bf16, tag="ag_in")
                    agsp = "Local" if "ag" in skip else "Shared"
                    tabA2 = dram_pool.tile(
                        [NCORES * SPL, TILE], bf16, tag="tabA", addr_space=agsp
                    )
                    tabB2 = dram_pool.tile(
                        [NCORES * (NPC - SPL), TILE],
                        bf16,
                        tag="tabB",
                        addr_space=agsp,
                    )
                gq = [0]
                for s, tiles in enumerate(plan.st_tiles):
                    nch = int(plan.st_nchunks[s])
                    goff = int(plan.st_chunk_off[s])
                    msg = msg_pool.tile([TILE, nch * TILE], bf16, tag="msg")
                    msg3 = msg[:].rearrange("p (c e) -> p c e", e=TILE)
                    s_sb = s_next if HOIST_S else gen_S(s)
                    g_live = "gather" not in skip
                    # pre-generate the next supertile's S (same for all layers)
                    if HOIST_S:
                        if s + 1 < plan.NST:
                            s_next = gen_S(s + 1)
                        elif not last or rep + 1 < repeats:
                            s_next = gen_S(0)
                    # gather calls split to <=1024 indices per call (SWDGE
                    # ring capacity)
                    GMAX = GATHER_CALL_IDX
                    tA = xtab1_p if layer == 0 else tabA
                    tB = xtab2_p if layer == 0 else tabB
                    if not g_live:
                        nc.gpsimd.dma_gather(
                            msg3[:, 0:1, :], tA[:],
                            idx_sb[:, goff * 8 : goff * 8 + 8],
                            num_idxs=TILE, num_idxs_reg=TILE, elem_size=TILE,
                        )
                    for t in tiles:
                        for h in range(2):
                            k = int((KA if h == 0 else KB)[t])
                            if k == 0 or not g_live:
                                continue
                            n = k * TILE
                            gchunk0 = int((plan.gbaseA if h == 0 else plan.gbaseB)[t])
                            lbase = int((plan.lbaseA if h == 0 else plan.lbaseB)[t])
                            coloff = gchunk0 * 8
                            in_ap = tA[:] if h == 0 else tB[:]
                            for c0 in range(0, n, GMAX):
                                nn = min(GMAX, n - c0)
                                out_ap = msg3[
                                    :,
                                    lbase + c0 // TILE : lbase + (c0 + nn) // TILE,
                                    :,
                                ]
                                nc.gpsimd.dma_gather(
                                    out_ap,
                                    in_ap,
                                    idx_sb[
                                        :, coloff + c0 // 16 : coloff + (c0 + nn) // 16
                                    ],
                                    num_idxs=nn,
                                    num_idxs_reg=nn,
                                    elem_size=TILE,
                                    queue_num=gq[0] % NUM_QUEUES,
                                    single_packet=SINGLE_PACKET,
                                )
                                gq[0] += 1
                    if not last and s == plan.NST - 4:
                        # T1 staging (tiles 0..SPL/128-1) is complete by now;
                        # the collective runs while later supertiles gather
                        allgather_part(ag_in2, tabA2, 0, SPL)
                    # per-tile accumulation, tile-major chunk order; the
                    # self-loop term dinv_i^2 (h W)_i enters the same PSUM as
                    # a W^T @ (h*dinv) matmul (contraction over features)
                    for t in tiles:
                        kA, kB = int(KA[t]), int(KB[t])
                        lA = int(plan.gbaseA[t] - goff)
                        lB = int(plan.gbaseB[t] - goff)
                        locs = [lA + j for j in range(kA)] + [lB + j for j in range(kB)]
                        if "mm" in skip:
                            locs = locs[:1]
                        psA = psA_pool.tile([D, TILE], f32, tag="psA")
                        if layer > 0:
                            nc.tensor.matmul(
                                psA[:],
                                w_sb[:, layer * D : (layer + 1) * D],
                                cur_D[:, t * TILE : (t + 1) * TILE],
                                start=True,
                                stop=(len(locs) == 0),
                            )
                        for ji, j in enumerate(locs):
                            nc.tensor.matmul(
                                psA[:],
                                msg3[:, j if g_live else 0, 0:D],
                                s_sb[:, (j if s_live else 0) * TILE : ((j if s_live else 0) + 1) * TILE],
                                start=(layer == 0 and ji == 0),
                                stop=(ji == len(locs) - 1),
                            )
                        if layer == 0:
                            # agg = sum dinv_src*x_src (+ self-loop dinv_i*x_i),
                            # then apply W0 (commuted past the aggregation)
                            agg = tmp_pool.tile([D, TILE], bf16, tag="agg")
                            nc.vector.tensor_tensor(
                                agg[:],
                                psA[:],
                                xTd_sb[:, t * TILE : (t + 1) * TILE],
                                ALU.add,
                            )
                            psA = psA_pool.tile([D, TILE], f32, tag="psA")
                            nc.tensor.matmul(
                                psA[:],
                                w_sb[:, 0:D],
                                agg[:],
                                start=True,
                                stop=True,
                            )
                        tmp = tmp_pool.tile([D, TILE], f32, tag="tmp")
                        nc.vector.tensor_tensor(
                            tmp[:], psA[:], dinvT_sb[:, t * TILE : (t + 1) * TILE], ALU.mult
                        )
                        nc.scalar.activation(
                            new_T[:, t * TILE : (t + 1) * TILE],
                            tmp[:],
                            AF.Relu,
                            bias=biasT_sb[:, layer : layer + 1],
                        )
                        if not last:
                            nc.vector.tensor_tensor(
                                new_D[:, t * TILE : (t + 1) * TILE],
                                new_T[:, t * TILE : (t + 1) * TILE],
                                dinvT_sb[:, t * TILE : (t + 1) * TILE],
                                ALU.mult,
                            )
                            transform_stage(t, layer + 1, new_T, ag_in2)
                        else:
                            classifier(t, new_T)
                if not last:
                    allgather_part(ag_in2, tabB2, SPL, NPC)
                    tabA, tabB = tabA2, tabB2
                    cur_T, cur_D = new_T, new_D
                else:
                    cur_T = new_T

    nc.compile()
    return nc


# ---------------------------------------------------------------------------
# in_maps assembly
# ---------------------------------------------------------------------------


def _in_maps(plan, W0, b0, W1, b1, W2, b2, W3, b3, Wl, bl):
    Ws = np.concatenate(
        [np.asarray(w, np.float32) for w in (W0, W1, W2, W3)], axis=1
    )  # [96, 4*96]
    biasT = np.stack(
        [np.asarray(b, np.float32) for b in (b0, b1, b2, b3)], axis=1
    )  # [96, 4]
    brep = np.tile(np.asarray(bl, np.float32)[None, :], (TILE, 1))  # [128, 4]
    wl = np.asarray(Wl, np.float32)
    maps = []
    for c in range(NCORES):
        pc = plan.per_core[c]
        maps.append(
            {
                "xtab1": plan.xtab1,
                "xtab2": plan.xtab2,
                "xTd": pc["xTd"],
                "idx": pc["idx"],
                "dstv": pc["dstv"],
                "iota": plan.iota,
                "dinvT": pc["dinvT"],
                "dinvown": pc["dinvown"],
                "biasT": biasT,
                "brep": brep,
                "W": Ws,
                "Wl": wl,
                "out": np.zeros((plan.NPC, 4), np.float32),
            }
        )
    return maps


# ---------------------------------------------------------------------------
# public entry point
# ---------------------------------------------------------------------------

_CACHE = {}


def _get_compiled(plan):
    return _build(plan)


def kernel(x, edge_index, W0, b0, W1, b1, W2, b2, W3, b3, Wl, bl):
    from concourse.bass_utils import run_bass_kernel_spmd

    x = np.asarray(x, np.float32)
    edge_index = np.asarray(edge_index, np.int64)
    plan = _prep(x, edge_index)
    nc = _get_compiled(plan)
    in_maps = _in_maps(plan, W0, b0, W1, b1, W2, b2, W3, b3, Wl, bl)
    res = run_bass_kernel_spmd(nc, in_maps, core_ids=list(range(NCORES)))
    out = np.concatenate([res.results[c]["out"] for c in range(NCORES)], axis=0)
    return out.astype(np.float32)

